# Initial kernel scaffold
#
"""Hamming-distance kernel for Trainium2 (8 NeuronCores, SPMD).

out[n, m] = mean_d(x[n, d] != y[m, d]),  x: (8192, 256), y: (8192, 256),
values are small integers 0..7 stored as float32.

Formulation: equality count as a one-hot GEMM (same as the reference):
    eq[n, m] = sum_{d, c} 1[x_nd == c] * 1[y_md == c]
One-hot values are exactly representable in fp8e4, and PSUM accumulates in
fp32 (max value 256 << 2^24), so the fp8 DoubleRow matmul is bit-exact.

Sharding: x rows split across 8 cores (1024 rows each), y replicated.
Each core computes a (1024, 8192) slice of the output.

Device-side pipeline per core:
  1. DMA x^T shard (256, 1024) and y^T (256, 8192) f32 (host supplies the
     transposed layout so DMA loads are contiguous; feature dim must sit on
     SBUF partitions for the matmul contraction).
  2. Encode one-hot k-major directly: for category c and d-half h, a single
     DVE tensor_scalar(is_equal) produces the 128-partition feature block
     f = (2c+h)*128 + p as an fp8 tile.  K = 8*256 = 2048 features.
  3. fp8 DoubleRow GEMM: out_tile[128, 512] accumulated over 8 k-pairs.
  4. ACT-engine eviction fused with the affine map 1 - eq/256.
"""

import numpy as np

import concourse.bass as bass
import concourse.mybir as mybir
import concourse.tile as tile
from concourse.bass_utils import run_bass_kernel_spmd

# Problem dims (hardcoded per contract).
N, M, D, C = 8192, 8192, 256, 8
N_CORES = 8
N_SH = N // N_CORES  # 1024 x-rows per core

P = 128
D_HALVES = D // P  # 2
KSUB = C * D_HALVES  # 16 k-subtiles of 128 features -> K = 2048
K_PAIRS = KSUB // 2  # 8 DoubleRow pairs (256 contracted per matmul)
M_CHUNK = 512  # output free-dim tile (one PSUM bank of f32)
M_CHUNKS = M // M_CHUNK  # 16
N_TILES = N_SH // P  # 8
M_GROUP = 4  # m-chunks per psum group (4 banks busy, 8 total)
M_GROUPS = M_CHUNKS // M_GROUP  # 4

FP8 = mybir.dt.float8e4
F32 = mybir.dt.float32


def _build_bass():
    nc = bass.Bass()

    xt_d = nc.dram_tensor("xt", [D, N_SH], F32, kind="ExternalInput")
    yt_d = nc.dram_tensor("yt", [D, M], F32, kind="ExternalInput")
    out_d = nc.dram_tensor("out", [N_SH, M], F32, kind="ExternalOutput")

    xt_r = xt_d.rearrange("(h p) n -> p h n", p=P)
    yt_r = yt_d.rearrange("(h p) m -> p h m", p=P)

    with tile.TileContext(nc) as tc:
        with (
            tc.tile_pool(name="xe", bufs=1) as xe_pool,
            tc.tile_pool(name="ye", bufs=M_CHUNKS) as ye_pool,
            tc.tile_pool(name="xraw", bufs=1) as xraw_pool,
            tc.tile_pool(name="yraw", bufs=3) as yraw_pool,
            tc.tile_pool(name="out", bufs=6) as out_pool,
            tc.tile_pool(name="psum", bufs=8, space="PSUM") as psum_pool,
        ):
            # ---- x side: load transposed shard, one-hot encode ----
            xt_sb = xraw_pool.tile([P, D_HALVES, N_SH], F32)
            nc.sync.dma_start(xt_sb[:], xt_r)
            xe = xe_pool.tile([P, KSUB, N_SH], FP8)
            for c in range(C):
                for h in range(D_HALVES):
                    nc.vector.tensor_scalar(
                        out=xe[:, 2 * c + h, :],
                        in0=xt_sb[:, h, :],
                        scalar1=float(c),
                        scalar2=None,
                        op0=mybir.AluOpType.is_equal,
                    )

            # ---- y side: per m-chunk, load transposed + encode ----
            ye_tiles = []
            for mc in range(M_CHUNKS):
                yt_sb = yraw_pool.tile([P, D_HALVES, M_CHUNK], F32)
                nc.sync.dma_start(
                    yt_sb[:], yt_r[:, :, mc * M_CHUNK : (mc + 1) * M_CHUNK]
                )
                ye_mc = ye_pool.tile([P, KSUB, M_CHUNK], FP8)
                for c in range(C):
                    for h in range(D_HALVES):
                        nc.vector.tensor_scalar(
                            out=ye_mc[:, 2 * c + h, :],
                            in0=yt_sb[:, h, :],
                            scalar1=float(c),
                            scalar2=None,
                            op0=mybir.AluOpType.is_equal,
                        )
                ye_tiles.append(ye_mc)

            # ---- GEMM + fused eviction ----
            for mg in range(M_GROUPS):
                for n in range(N_TILES):
                    psum_tiles = [
                        psum_pool.tile([P, M_CHUNK], F32) for _ in range(M_GROUP)
                    ]
                    for kp in range(K_PAIRS):
                        lhsT = xe[:, 2 * kp : 2 * kp + 2, n * P : (n + 1) * P]
                        for j in range(M_GROUP):
                            nc.tensor.matmul(
                                psum_tiles[j][:],
                                lhsT,
                                ye_tiles[mg * M_GROUP + j][:, 2 * kp : 2 * kp + 2, :],
                                start=(kp == 0),
                                stop=(kp == K_PAIRS - 1),
                                perf_mode=mybir.MatmulPerfMode.DoubleRow,
                            )
                    for j in range(M_GROUP):
                        ot = out_pool.tile([P, M_CHUNK], F32)
                        nc.scalar.activation(
                            ot[:],
                            psum_tiles[j][:],
                            mybir.ActivationFunctionType.Copy,
                            bias=1.0,
                            scale=-1.0 / float(D),
                        )
                        mc = mg * M_GROUP + j
                        nc.sync.dma_start(
                            out_d[
                                n * P : (n + 1) * P,
                                mc * M_CHUNK : (mc + 1) * M_CHUNK,
                            ],
                            ot[:],
                        )
    return nc


_NC_CACHE = None


def kernel(x: np.ndarray, y: np.ndarray, _trace: bool = False):
    global _NC_CACHE
    x = np.asarray(x, dtype=np.float32)
    y = np.asarray(y, dtype=np.float32)
    assert x.shape == (N, D) and y.shape == (M, D)

    if _NC_CACHE is None:
        _NC_CACHE = _build_bass()
    nc = _NC_CACHE

    yt = np.ascontiguousarray(y.T)  # (256, 8192)
    in_maps = []
    for i in range(N_CORES):
        xt_i = np.ascontiguousarray(x[i * N_SH : (i + 1) * N_SH].T)  # (256, 1024)
        in_maps.append({"xt": xt_i, "yt": yt})

    res = run_bass_kernel_spmd(
        nc, in_maps, core_ids=list(range(N_CORES)), trace=_trace
    )
    out = np.concatenate([r["out"] for r in res.results], axis=0)
    if _trace:
        return out, res
    return out


# revision 12
# speedup vs baseline: 1.0266x; 1.0266x over previous
"""Hamming-distance kernel for Trainium2 (8 NeuronCores, SPMD).

out[n, m] = mean_d(x[n, d] != y[m, d]),  x: (8192, 256), y: (8192, 256),
values are small integers 0..7 stored as float32.

Formulation: equality count as a one-hot GEMM (same as the reference):
    eq[n, m] = sum_{d, c} 1[x_nd == c] * 1[y_md == c]
One-hot values are exactly representable in fp8e4, and PSUM accumulates in
fp32 (max value 256 << 2^24), so the fp8 DoubleRow matmul is bit-exact.

Sharding: x rows split across 8 cores (1024 rows each), y replicated.
Each core computes a (1024, 8192) slice of the output.

Device-side pipeline per core:
  1. DMA x^T shard (256, 1024) and y^T (256, 8192) f32 into fresh SBUF
     slots (host supplies the transposed layout so loads are contiguous;
     the feature dim must sit on SBUF partitions for the contraction).
     All load DMAs write never-reused slots so each needs at most one sem
     wait (the DMA ISA has a single wait slot).
  2. One-hot encode k-major: for category c and d-half h, a single DVE
     tensor_scalar(is_equal) yields feature block f = (2c+h)*128 + p as an
     fp8 tile.  K = 8*256 = 2048.  y is encoded per m-group into a ring of
     fp8 chunk tiles so DVE encode overlaps the previous group's matmuls.
  3. fp8 DoubleRow GEMM: psum[128, 512] accumulated over 8 k-pairs.
  4. ACT-engine PSUM eviction fused with the affine map 1 - eq/256.
"""

import numpy as np

import concourse.bacc as bacc
import concourse.bass as bass
import concourse.mybir as mybir
import concourse.tile as tile
from concourse.bass_utils import run_bass_kernel_spmd

# Problem dims (hardcoded per contract).
N, M, D, C = 8192, 8192, 256, 8
N_CORES = 8
N_SH = N // N_CORES  # 1024 x-rows per core

P = 128
D_HALVES = D // P  # 2
KSUB = C * D_HALVES  # 16 k-subtiles of 128 features -> K = 2048
K_PAIRS = KSUB // 2  # 8 DoubleRow pairs (256 contracted per matmul)
M_CHUNK = 512  # output free-dim tile (one PSUM bank of f32)
M_CHUNKS = M // M_CHUNK  # 16
N_TILES = N_SH // P  # 8
M_GROUP = 4  # m-chunks per psum group (4 banks busy, 8 total)
M_GROUPS = M_CHUNKS // M_GROUP  # 4
MG_COLS = M_GROUP * M_CHUNK  # 2048 m-columns per group

FP8 = mybir.dt.float8e4
F32 = mybir.dt.float32


def _build_bass(repeats: int = 1):
    # Bacc (not raw Bass): its compile() legalizes multi-semaphore waits
    # into EventSemaphore instructions (HW allows 1 wait per instruction).
    nc = bacc.Bacc(
        "TRN2", target_bir_lowering=False, debug=False, num_devices=N_CORES
    )

    xt_d = nc.dram_tensor("xt", [D, N_SH], F32, kind="ExternalInput")
    yt_d = nc.dram_tensor("yt", [D, M], F32, kind="ExternalInput")
    # Blocked output layout: block (n, mc) is one contiguous 128x512 f32
    # region, so store DMAs are interval-disjoint (no false WAW chains that
    # would exceed the DMA ISA's single sem-wait slot) and fully contiguous.
    # The host de-blocks with a transpose+reshape.
    out_d = nc.dram_tensor(
        "out", [N_TILES, M_CHUNKS, P, M_CHUNK], F32, kind="ExternalOutput"
    )

    xt_r = xt_d.rearrange("(h p) n -> p h n", p=P)
    yt_r = yt_d.rearrange("(h p) m -> p h m", p=P)

    with tile.TileContext(nc) as tc:
        with (
            tc.tile_pool(name="xe", bufs=1) as xe_pool,
            tc.tile_pool(name="ye", bufs=2 * M_GROUP) as ye_pool,
            tc.tile_pool(name="xraw", bufs=1) as xraw_pool,
            tc.tile_pool(name="yraw", bufs=M_GROUPS) as yraw_pool,
            tc.tile_pool(name="out", bufs=8) as out_pool,
            tc.tile_pool(name="psum", bufs=8, space="PSUM") as psum_pool,
        ):
            # ---- raw loads: all into fresh slots, so zero/one sem wait ----
            xt_sb = xraw_pool.tile([P, D_HALVES, N_SH], F32)
            nc.sync.dma_start(xt_sb[:], xt_r)
            yraw_tiles = []
            for mg in range(M_GROUPS):
                yt_sb = yraw_pool.tile([P, D_HALVES, MG_COLS], F32, name="yt_sb")
                nc.sync.dma_start(
                    yt_sb[:], yt_r[:, :, mg * MG_COLS : (mg + 1) * MG_COLS]
                )
                yraw_tiles.append(yt_sb)

            # ---- x one-hot (16 DVE compares) ----
            xe = xe_pool.tile([P, KSUB, N_SH], FP8)
            for c in range(C):
                for h in range(D_HALVES):
                    nc.vector.tensor_scalar(
                        out=xe[:, 2 * c + h, :],
                        in0=xt_sb[:, h, :],
                        scalar1=float(c),
                        scalar2=None,
                        op0=mybir.AluOpType.is_equal,
                    )

            def _one_pass():
                for mg in range(M_GROUPS):
                    # y one-hot for this m-group (ring of fp8 chunk tiles;
                    # encode of group g+1 overlaps matmuls of group g)
                    ye_tiles = []
                    for j in range(M_GROUP):
                        ye_mc = ye_pool.tile([P, KSUB, M_CHUNK], FP8, name="ye_mc")
                        for c in range(C):
                            for h in range(D_HALVES):
                                nc.vector.tensor_scalar(
                                    out=ye_mc[:, 2 * c + h, :],
                                    in0=yraw_tiles[mg][
                                        :, h, j * M_CHUNK : (j + 1) * M_CHUNK
                                    ],
                                    scalar1=float(c),
                                    scalar2=None,
                                    op0=mybir.AluOpType.is_equal,
                                )
                        ye_tiles.append(ye_mc)

                    for n in range(N_TILES):
                        psum_tiles = [
                            psum_pool.tile([P, M_CHUNK], F32, name="psum")
                            for _ in range(M_GROUP)
                        ]
                        for kp in range(K_PAIRS):
                            lhsT = xe[:, 2 * kp : 2 * kp + 2, n * P : (n + 1) * P]
                            for j in range(M_GROUP):
                                nc.tensor.matmul(
                                    psum_tiles[j][:],
                                    lhsT,
                                    ye_tiles[j][:, 2 * kp : 2 * kp + 2, :],
                                    start=(kp == 0),
                                    stop=(kp == K_PAIRS - 1),
                                    perf_mode=mybir.MatmulPerfMode.DoubleRow,
                                )
                        for j in range(M_GROUP):
                            ot = out_pool.tile([P, M_CHUNK], F32, name="ot")
                            nc.scalar.activation(
                                ot[:],
                                psum_tiles[j][:],
                                mybir.ActivationFunctionType.Copy,
                                bias=1.0,
                                scale=-1.0 / float(D),
                            )
                            mc = mg * M_GROUP + j
                            nc.sync.dma_start(out_d[n, mc], ot[:])

            if repeats == 1:
                _one_pass()
            else:
                # device-side repeat loop, used only for wall-clock timing
                with tc.For_i(0, repeats, 1):
                    _one_pass()
    nc.compile()
    return nc


_NC_CACHE = {}


def _get_nc(repeats: int = 1):
    if repeats not in _NC_CACHE:
        _NC_CACHE[repeats] = _build_bass(repeats)
    return _NC_CACHE[repeats]


def _make_in_maps(x: np.ndarray, y: np.ndarray):
    yt = np.ascontiguousarray(y.T)  # (256, 8192)
    in_maps = []
    for i in range(N_CORES):
        xt_i = np.ascontiguousarray(x[i * N_SH : (i + 1) * N_SH].T)  # (256, 1024)
        in_maps.append({"xt": xt_i, "yt": yt})
    return in_maps


def kernel(x: np.ndarray, y: np.ndarray) -> np.ndarray:
    x = np.asarray(x, dtype=np.float32)
    y = np.asarray(y, dtype=np.float32)
    assert x.shape == (N, D) and y.shape == (M, D)

    nc = _get_nc(1)
    in_maps = _make_in_maps(x, y)
    res = run_bass_kernel_spmd(nc, in_maps, core_ids=list(range(N_CORES)))
    return np.concatenate(
        [_deblock(r["out"]) for r in res.results], axis=0
    )


def _deblock(blocked: np.ndarray) -> np.ndarray:
    # (N_TILES, M_CHUNKS, P, M_CHUNK) -> (N_SH, M)
    return np.ascontiguousarray(
        blocked.transpose(0, 2, 1, 3).reshape(N_SH, M)
    )
